# revision 23
# baseline (speedup 1.0000x reference)
"""Trainium2 Bass kernel for nn_MultiHeadAttention_9131100471662.

Cross-attention with memory tokens, dual softmax (rows+columns of the
affinity matrix), head-mean, masked tokens.

v3 strategy:
  - Data-parallel over batch: 16 batches -> 8 cores x 2 batches.
  - Host-side mask compaction (exact), T=288 fixed slots.
  - Per batch, two affinity orientations:
      d=0: e0_h[x,y] = exp(aff), ScalarE singles with accum_out -> den_Y
           (softmax-over-y dens). One-hot PE matvecs partition-sum the
           e0 tiles -> den_X (softmax-over-x dens) -> rcp -> ln(rcp).
      d=1: e1_h[y,x] = exp(aff + ln(rcp_X[y]))  -- the per-partition bias
           PRE-NORMALIZES the tiles, so the head-sum is a plain
           tensor_tensor add tree on DVE (2x perf mode) instead of the
           1x scalar_tensor_tensor chain. No accum_out needed.
    (USE_LNBIAS=False falls back to paired d=1 exps + STT chains.)
  - d=0 normalize: DVE scalar_tensor_tensor chains with rcp_Y.
  - Emission order = engine priority: ScalarE (exp) is the global
    bottleneck (~100us); PE work (projections of the next pass, dens
    matvecs, transposes, output matmuls) is woven between affinity
    groups at fine grain so neither ScalarE nor PE idles (PE drops to a
    1.2 GHz p-state when idle, doubling matmul cost).
  - PSUM rings: "af2" [128,2,512] f32 x2 (affinity pairs), "pj" [128,512]
    f32 x2 (projection singles + output matmuls), "tp" [128,2,512] bf16 x1
    (transposes), "big" [128,512] f32 x1 (dens matvec accumulator) = 8 banks.
  - 1/HEADS head-mean folded into host-side memory matrices.
"""

import numpy as np

import bass_rust
import concourse.bass as bass
import concourse.mybir as mybir
from concourse.tile import TileContext

B = 16
SEQ = 512
HIDDEN = 1024
HEADS = 16
MEM = 2
DH = 64
NCORES = 8
BPC = 2
T_DEFAULT = 288
F32 = mybir.dt.float32
BF16 = mybir.dt.bfloat16
F16 = mybir.dt.float16

PROJ_DT = F16
E_DT = BF16
A_DT = BF16
MEM_DT = BF16
OUT_DT = BF16

USE_LNBIAS = False


def _chunks(T):
    out = []
    o = 0
    while o < T:
        w = min(128, T - o)
        out.append((o, w))
        o += w
    return out


def _patched_drain_and_barrier(self, tick_clock, wait_clock):
    # Workaround: this walrus build rejects a Drain carrying >1 sem waits.
    nc = self.nc
    drain_inst = nc.sync.drain()
    wait_clock.add_sem_waits(
        drain_inst.ins, bass_rust.ScopedClock({None: tick_clock.global_clock})
    )
    inst = drain_inst.ins
    si = inst.sync_info
    waits = list(si.on_wait) if si and si.on_wait else []
    si.on_wait = []
    name2sem = {s.name: s for s in self.sems.allocated().values()}
    for w in waits:
        assert w.wait_mode == "sem-ge-imm", w
        nc.sync.wait_ge(name2sem[w.ant_name], w.wait_value)
    nc.all_engine_barrier()
    popped = nc._tile_sem_poison_stack.pop()
    assert popped is self._sem_poison
    nc.clear_and_free_semaphores(list(self.sems.allocated().values()))
    nc.all_engine_barrier()


TileContext._drain_and_barrier = _patched_drain_and_barrier


def split_excess_waits(nc, cap=1):
    """Hoist >cap sem waits per instruction onto injected NoOps."""
    for f in nc.m.functions:
        for bb in f.blocks:
            newlist, changed = [], False
            for inst in bb.instructions:
                si = inst.sync_info
                waits = list(si.on_wait) if si and si.on_wait else []
                if len(waits) > cap:
                    changed = True
                    for w in waits[:-cap]:
                        nop = mybir.InstNoOp(
                            name=nc.get_next_instruction_name(), ins=[], outs=[])
                        nop.engine = inst.engine
                        nop.sync_info = mybir.SyncInfo(on_wait=[w], on_update=[])
                        nc.register_instruction(nop, overwrite=True)
                        newlist.append(nop)
                    si.on_wait = waits[-cap:]
                newlist.append(inst)
            if changed:
                bb.instructions = newlist


def build_nc(T=T_DEFAULT):
    CH = _chunks(T)
    NT = len(CH)
    nc = bass.Bass()
    p = {}
    p["wxT"] = nc.declare_dram_parameter("wxT", [128, 8, HIDDEN], PROJ_DT, isOutput=False)
    p["wyT"] = nc.declare_dram_parameter("wyT", [128, 8, HIDDEN], PROJ_DT, isOutput=False)
    p["ident"] = nc.declare_dram_parameter("ident", [128, 128], F32, isOutput=False)
    for s in range(BPC):
        p[f"xT{s}"] = nc.declare_dram_parameter(f"xT{s}", [128, 8, T], PROJ_DT, isOutput=False)
        p[f"yT{s}"] = nc.declare_dram_parameter(f"yT{s}", [128, 8, T], PROJ_DT, isOutput=False)
        p[f"xc{s}"] = nc.declare_dram_parameter(f"xc{s}", [128, NT, HIDDEN], MEM_DT, isOutput=False)
        p[f"yc{s}"] = nc.declare_dram_parameter(f"yc{s}", [128, NT, HIDDEN], MEM_DT, isOutput=False)
    p["corr"] = nc.declare_dram_parameter("corr", [128, 2 * BPC], F32, isOutput=False)
    for s in range(BPC):
        # outputs transposed, per-128-column chunk: [hc, part, T]
        p[f"xiyT{s}"] = nc.declare_dram_parameter(f"xiyT{s}", [8, 128, T], OUT_DT, isOutput=True)
        p[f"yixT{s}"] = nc.declare_dram_parameter(f"yixT{s}", [8, 128, T], OUT_DT, isOutput=True)

    with TileContext(nc, pool_alloc_mode="queue") as tc:
        import contextlib
        with contextlib.ExitStack() as ctx:
            cpool = ctx.enter_context(tc.tile_pool(name="consts", bufs=1))
            projpool = ctx.enter_context(tc.tile_pool(name="proj", bufs=1))
            psum = ctx.enter_context(tc.tile_pool(name="psum", bufs=1, space="PSUM"))
            epool = ctx.enter_context(tc.tile_pool(name="epool", bufs=1))
            apool = ctx.enter_context(tc.tile_pool(name="apool", bufs=1))
            smallpool = ctx.enter_context(tc.tile_pool(name="small", bufs=1))
            xcpool = ctx.enter_context(tc.tile_pool(name="xcpool", bufs=1))
            w_scope = contextlib.ExitStack()
            wpool = w_scope.enter_context(tc.tile_pool(name="weights", bufs=1))
            inpool = w_scope.enter_context(tc.tile_pool(name="inputs", bufs=1))

            _c = {}

            def preload_exp_table():
                t_ = cpool.tile([128, 16], F32, name="dummy")
                nc.vector.memset(t_[:, :], 0.0)
                nc.scalar.activation(t_[:, :], t_[:, :],
                                     mybir.ActivationFunctionType.Exp)
                if USE_LNBIAS:
                    nc.vector.memset(t_[:, :], 1.0)
                    nc.scalar.activation(t_[:, :], t_[:, :],
                                         mybir.ActivationFunctionType.Ln)
                # PE p-state warmup during the input-DMA wait: ~3us of dummy
                # matmuls ramps the tensor engine to 2.4 GHz before the first
                # projection (cold PE runs at 0.65-1.2 GHz).
                wb = cpool.tile([128, 448], BF16, name="warm")
                nc.vector.memset(wb[:, :], 0.0)
                wp = psum.tile([128, 512], F32, name="warm_ps", tag="pj", bufs=2)
                for i in range(18):
                    nc.tensor.matmul(wp[0:64, 0:448], wb[:, 0:64], wb[:, :],
                                     start=(i == 0), stop=(i == 17))
                nc.vector.tensor_copy(t_[0:1, 0:1], wp[0:1, 0:1])

            def load_consts():
                ident_sb = cpool.tile([128, 128], F32, name="ident_sb")
                nc.sync.dma_start(out=ident_sb[:, :], in_=p["ident"][:, :])
                identb_sb = cpool.tile([128, 128], A_DT, name="identb_sb")
                nc.vector.tensor_copy(identb_sb[:, :], ident_sb[:, :])
                corr_sb = cpool.tile([128, 2 * BPC], F32, name="corr_sb")
                nc.sync.dma_start(out=corr_sb[:, :], in_=p["corr"][:, :])
                for s_ in range(BPC):
                    for ci, side in enumerate(("x", "y")):
                        _c[f"cor{side}{s_}"] = corr_sb[:, 2 * s_ + ci:2 * s_ + ci + 1]
                oh = cpool.tile([128, HEADS, HEADS], E_DT, name="onehot_sb")
                nc.vector.memset(oh[:, :, :], 0.0)
                for h in range(HEADS):
                    nc.vector.memset(oh[:, h, h:h + 1], 1.0)
                _c["onehot"] = oh
                _c["ident"], _c["identb"] = ident_sb, identb_sb

            w_sb, tT_sb = {}, {}
            w_first = {}

            def load_w_first(side, ot=0):
                wname = "wxT" if side == "x" else "wyT"
                t_ = wpool.tile([128, 8, 128], PROJ_DT, name=f"wf{side}{ot}",
                                tag=f"wf{side}{ot}")
                nc.sync.dma_start(out=t_[:, :, :],
                                  in_=p[wname][:, :, ot * 128:(ot + 1) * 128])
                w_first[(side, ot)] = t_

            def load_w_half(side, hf):
                wname = "wxT" if side == "x" else "wyT"
                t_ = wpool.tile([128, 8, 512], PROJ_DT, name=f"w{side}{hf}",
                                tag=f"w{side}{hf}")
                nc.sync.dma_start(out=t_[:, :, :],
                                  in_=p[wname][:, :, hf * 512:(hf + 1) * 512])
                for kt in range(8):
                    w_sb[(side, kt, hf)] = t_[:, kt, :]

            def load_tT(s, side):
                t_ = inpool.tile([128, 8, T], PROJ_DT, name=f"tT{side}{s}",
                                 tag=f"tT{side}{s}")
                nc.sync.dma_start(out=t_[:, :, :], in_=p[f"{side}T{s}"][:, :, :])
                for kt in range(8):
                    tT_sb[(s, side, kt)] = t_[:, kt, :]

            def load_mem(s):
                for side in ("x", "y"):
                    t_ = xcpool.tile([128, NT, HIDDEN], MEM_DT,
                                     name=f"mem{side}{s}", tag=f"mem{side}", bufs=1)
                    nc.sync.dma_start(out=t_[:, :, :], in_=p[f"{side}c{s}"][:, :, :])
                    for kt in range(NT):
                        mem_sb[(s, side, kt)] = t_[:, kt, :]

            proj_sb, mem_sb = {}, {}
            e_sb, den_sb, rcp_sb, lnr_sb, a_sb, at_sb, rs_ps = {}, {}, {}, {}, {}, {}, {}

            def _w_slice(side, ot, kt):
                if (side, ot) in w_first:
                    return w_first[(side, ot)][:, kt, :]
                return w_sb[(side, kt, ot // 4)][:, (ot % 4) * 128:(ot % 4 + 1) * 128]

            def emit_proj_single(s, side, ot):
                ptf = psum.tile([128, 512], F32, name="pj_ps", tag="pj", bufs=2)
                pt = ptf[:, 0:T]
                for kt in range(8):
                    nc.tensor.matmul(
                        pt, _w_slice(side, ot, kt), tT_sb[(s, side, kt)][:, :],
                        start=(kt == 0), stop=(kt == 7),
                    )
                st = projpool.tile([128, T], PROJ_DT, name=f"pj{s}{side}{ot}",
                                   tag=f"pj{s}{side}{ot}")
                nc.vector.tensor_copy(st[:, :], pt)
                proj_sb[(s, side, ot)] = st

            def alloc_den(s):
                for mt in range(NT):
                    den_sb[(s, 0, mt)] = smallpool.tile(
                        [128, HEADS], F32, name=f"den{s}0{mt}", tag=f"den0{mt}", bufs=2)

            def emit_aff(s, d, ot, mt):
                """Affinity head-pair (2ot, 2ot+1), stationary chunk mt.
                d=0: singles + accum_out (den_Y).  d=1 (LNBIAS): singles with
                bias=ln(rcp_X); else one paired exp."""
                stat_side, mov_side = ("x", "y") if d == 0 else ("y", "x")
                lo_c, w_c = CH[mt]
                stat = proj_sb[(s, stat_side, ot)]
                mov = proj_sb[(s, mov_side, ot)]
                af = psum.tile([128, 2, 512], F32, name="af_ps", tag="af2", bufs=2)
                for half in range(2):
                    lo = 64 * half
                    nc.tensor.matmul(
                        af[0:w_c, half, 0:T],
                        stat[lo:lo + 64, lo_c:lo_c + w_c],
                        mov[lo:lo + 64, :],
                        start=True, stop=True,
                    )
                ep = epool.tile([128, 2, T], E_DT, name="e_t", tag=f"e{d}",
                                bufs=(48 if d == 0 else 27))
                if d == 0:
                    for half in range(2):
                        h = 2 * ot + half
                        nc.scalar.activation(
                            ep[0:w_c, half, :], af[0:w_c, half, 0:T],
                            mybir.ActivationFunctionType.Exp,
                            accum_out=den_sb[(s, 0, mt)][0:w_c, h:h + 1],
                        )
                elif USE_LNBIAS:
                    for half in range(2):
                        h = 2 * ot + half
                        nc.scalar.activation(
                            ep[0:w_c, half, :], af[0:w_c, half, 0:T],
                            mybir.ActivationFunctionType.Exp,
                            bias=lnr_sb[(s, mt)][0:w_c, h:h + 1],
                        )
                else:
                    nc.scalar.activation(
                        ep[0:w_c, :, :], af[0:w_c, :, 0:T],
                        mybir.ActivationFunctionType.Exp,
                    )
                e_sb[(s, d, 2 * ot, mt)] = ep[:, 0, :]
                e_sb[(s, d, 2 * ot + 1, mt)] = ep[:, 1, :]

            def emit_mv(s, heads):
                """One-hot matvecs: partition-sum e0 tiles of `heads` over all
                chunks into rs[16, T] (accumulating across calls)."""
                rs = rs_ps.setdefault(
                    s, psum.tile([128, 512], F32, name="rs_ps", tag="big", bufs=1))
                for h in heads:
                    for chunk, (klo, kw) in enumerate(CH):
                        nc.tensor.matmul(
                            rs[0:16, 0:T],
                            _c["onehot"][0:kw, h, :],
                            e_sb[(s, 0, h, chunk)][0:kw, :],
                            start=(h == 0 and chunk == 0),
                            stop=(h == HEADS - 1 and chunk == NT - 1),
                            skip_group_check=True,
                        )

            def emit_mv_finalize(s):
                """rs -> per-y-chunk rcp_X (and ln(rcp_X) for the d=1 bias)."""
                rs = rs_ps[s]
                rssb = smallpool.tile([16, T], F32, name=f"rssb{s}", tag="rssb", bufs=2)
                nc.vector.tensor_copy(rssb[:, :], rs[0:16, 0:T])
                corr = _c[f"corx{s}"]
                for mt, (lo_c, w_c) in enumerate(CH):
                    dps = psum.tile([128, 512], F32, name="dps", tag="big", bufs=1)
                    nc.tensor.transpose(dps[0:w_c, 0:16], rssb[:, lo_c:lo_c + w_c],
                                        _c["ident"][0:16, 0:16])
                    nc.vector.tensor_scalar_sub(
                        dps[0:w_c, 0:16], dps[0:w_c, 0:16], corr[0:w_c, 0:1])
                    rcp = smallpool.tile([128, HEADS], F32, name=f"rcp{s}1{mt}",
                                         tag=f"rcp1{mt}", bufs=2)
                    nc.vector.reciprocal(rcp[0:w_c, :], dps[0:w_c, 0:16])
                    rcp_sb[(s, 1, mt)] = rcp
                    if USE_LNBIAS:
                        lnr = smallpool.tile([128, HEADS], F32, name=f"lnr{s}{mt}",
                                             tag=f"lnr{mt}", bufs=2)
                        nc.scalar.activation(lnr[0:w_c, :], rcp[0:w_c, :],
                                             mybir.ActivationFunctionType.Ln)
                        lnr_sb[(s, mt)] = lnr

            def norm_steps(s, d, mt):
                """Generator of DVE steps for a normalize chain (sliced so
                the emission loop can weave copies between steps)."""
                lo_c, w_c = CH[mt]
                if d == 0:
                    den = den_sb[(s, 0, mt)]
                    corr = _c[f"cory{s}"]
                    nc.vector.tensor_scalar_sub(den[0:w_c, :], den[0:w_c, :],
                                                corr[0:w_c, 0:1])
                    rcp = smallpool.tile([128, HEADS], F32, name=f"rcp{s}0{mt}",
                                         tag=f"rcp0{mt}", bufs=2)
                    nc.vector.reciprocal(rcp[0:w_c, :], den[0:w_c, :])
                else:
                    rcp = rcp_sb[(s, 1, mt)]
                es = [e_sb[(s, d, h, mt)] for h in range(HEADS)]
                # scale each head tile in place (4x DVE perf mode), then
                # tree-accumulate across heads on the DMA engines
                # (accum_op=add) -- takes the 15-op reduction off DVE.
                for h in range(HEADS):
                    nc.vector.tensor_scalar_mul(
                        es[h][0:w_c, :], es[h][0:w_c, :], rcp[0:w_c, h:h + 1])
                    if h % 4 == 3:
                        yield
                for step in (1, 2, 4, 8):
                    for h0 in range(0, HEADS, 2 * step):
                        nc.gpsimd.dma_start(
                            out=es[h0][0:w_c, :], in_=es[h0 + step][0:w_c, :],
                            accum_op=mybir.AluOpType.add)
                    yield
                a_sb[(s, d, mt)] = es[0]

            def emit_pair_add(s, ot, mt):
                """Tree level 0 for d=1: e[2ot] += e[2ot+1] (2x-mode add)."""
                lo_c, w_c = CH[mt]
                a = e_sb[(s, 1, 2 * ot, mt)]
                b = e_sb[(s, 1, 2 * ot + 1, mt)]
                nc.vector.tensor_tensor(out=a[0:w_c, :], in0=a[0:w_c, :],
                                        in1=b[0:w_c, :], op=mybir.AluOpType.add)

            def emit_tree_tail(s, mt):
                lo_c, w_c = CH[mt]
                if USE_LNBIAS:
                    for step in (2, 4, 8):
                        for h0 in range(0, HEADS, 2 * step):
                            a = e_sb[(s, 1, h0, mt)]
                            b = e_sb[(s, 1, h0 + step, mt)]
                            nc.vector.tensor_tensor(
                                out=a[0:w_c, :], in0=a[0:w_c, :],
                                in1=b[0:w_c, :], op=mybir.AluOpType.add)
                    a_sb[(s, 1, mt)] = e_sb[(s, 1, 0, mt)]
                else:
                    rcp = rcp_sb[(s, 1, mt)]
                    es = [e_sb[(s, 1, h, mt)] for h in range(HEADS)]
                    a = apool.tile([128, T], A_DT, name=f"a{s}1{mt}", tag=f"a1{mt}", bufs=2)
                    nc.vector.tensor_scalar_mul(a[0:w_c, :], es[0][0:w_c, :], rcp[0:w_c, 0:1])
                    for h in range(1, HEADS):
                        nc.vector.scalar_tensor_tensor(
                            out=a[0:w_c, :], in0=es[h][0:w_c, :],
                            scalar=rcp[0:w_c, h:h + 1], in1=a[0:w_c, :],
                            op0=mybir.AluOpType.mult, op1=mybir.AluOpType.add)
                    a_sb[(s, 1, mt)] = a

            def emit_transpose(s, d, act_copy=False):
                cpy = nc.scalar.copy if act_copy else nc.vector.tensor_copy
                slotA = psum.tile([128, 2, 512], A_DT, name="tp_ps", tag="tp", bufs=1)
                for kt in range(2):
                    klo, kw = CH[kt]
                    for mt, (mlo, mw) in enumerate(CH):
                        nc.tensor.transpose(
                            slotA[:, kt, :][0:kw, mlo:mlo + mw],
                            a_sb[(s, d, mt)][0:mw, klo:klo + kw],
                            _c["identb"][0:mw, 0:mw],
                        )
                stAB = apool.tile([128, 2, T], A_DT, name=f"atp{s}{d}", tag=f"atp{d}", bufs=2)
                cpy(stAB[:, :, :], slotA[:, :, 0:T])
                slotB = psum.tile([128, 2, 512], A_DT, name="tp_ps", tag="tp", bufs=1)
                klo, kw = CH[2]
                for mt, (mlo, mw) in enumerate(CH):
                    nc.tensor.transpose(
                        slotB[:, 0, :][0:kw, mlo:mlo + mw],
                        a_sb[(s, d, mt)][0:mw, klo:klo + kw],
                        _c["identb"][0:mw, 0:mw],
                    )
                st2 = apool.tile([128, T], A_DT, name=f"at2{s}{d}", tag=f"at2{d}", bufs=2)
                cpy(st2[0:kw, :], slotB[0:kw, 0, 0:T])
                at_sb[(s, d, 0)] = stAB[:, 0, :]
                at_sb[(s, d, 1)] = stAB[:, 1, :]
                at_sb[(s, d, 2)] = st2

            def emit_output_hc(s, d, hc, act_copy=False):
                cpy = nc.scalar.copy if act_copy else nc.vector.tensor_copy
                rhs_side, oname = (("y", f"yixT{s}"), ("x", f"xiyT{s}"))[d]
                opf = psum.tile([128, 512], F32, name="out_ps", tag="pj", bufs=2)
                op = opf[:, 0:T]
                for kt, (klo, kw) in enumerate(CH):
                    nc.tensor.matmul(
                        op,
                        mem_sb[(s, rhs_side, kt)][0:kw, hc * 128:(hc + 1) * 128],
                        at_sb[(s, d, kt)][0:kw, :],
                        start=(kt == 0), stop=(kt == NT - 1),
                    )
                ost = smallpool.tile([128, T], OUT_DT, name="ost", tag="ost", bufs=4)
                cpy(ost[:, :], op)
                nc.sync.dma_start(out=p[oname][hc, :, :], in_=ost[:, :])

            # ================= schedule =================
            load_tT(0, "x")
            load_tT(0, "y")
            load_w_first("x")
            load_w_first("y")
            preload_exp_table()
            alloc_den(0)

            # ---- P1: pass (0,0) ot-major; weave proj(0) lag-1, mv(0) lag-1
            emit_proj_single(0, "x", 0)
            emit_proj_single(0, "y", 0)
            load_w_half("x", 0)
            load_w_half("y", 0)
            for ot in range(8):
                emit_aff(0, 0, ot, 0)
                emit_aff(0, 0, ot, 1)
                if ot < 7:
                    emit_proj_single(0, "x", ot + 1)
                emit_aff(0, 0, ot, 2)
                if ot < 7:
                    emit_proj_single(0, "y", ot + 1)
                else:
                    emit_proj_single(1, "x", 0)
                    emit_proj_single(1, "y", 0)
                if ot == 0:
                    load_w_half("x", 1)
                    load_w_half("y", 1)
                    load_consts()
                if ot == 2:
                    load_tT(1, "x")
                    load_tT(1, "y")
                if ot == 4:
                    load_mem(0)
                if ot == 6:
                    load_mem(1)
                if ot > 0:
                    emit_mv(0, (2 * ot - 2, 2 * ot - 1))
            emit_mv(0, (14, 15))
            emit_mv_finalize(0)

            # ---- P2: pass (1,0) ot-major; weave proj(1) lag-1, mv(1) lag-1,
            # and the norm(0,0) STT chains (sliced) on DVE.
            alloc_den(1)
            n00 = [norm_steps(0, 0, mt) for mt in range(NT)]

            def pump(gens, k=1):
                for _ in range(k):
                    for g in list(gens):
                        try:
                            next(g)
                            break
                        except StopIteration:
                            gens.remove(g)

            for ot in range(8):
                emit_aff(1, 0, ot, 0)
                emit_aff(1, 0, ot, 1)
                if ot < 7:
                    emit_proj_single(1, "x", ot + 1)
                emit_aff(1, 0, ot, 2)
                if ot < 7:
                    emit_proj_single(1, "y", ot + 1)
                if ot > 0:
                    emit_mv(1, (2 * ot - 2, 2 * ot - 1))
                pump(n00, 2)
            emit_mv(1, (14, 15))
            emit_mv_finalize(1)
            pump(n00, 99)
            w_scope.close()

            # ---- P3: pass (0,1) mt-major; weave transpose(0,0)+output(0,0)
            # (copies on the then-slack ScalarE) and the norm(0,1) chains
            # with a one-chunk lag.
            dq = []
            out_q = [(0, 0, hc) for hc in range(8)]
            for mt in range(NT):
                for ot in range(8):
                    emit_aff(0, 1, ot, mt)
                    if mt == 0 and ot == 1:
                        emit_transpose(0, 0)
                    if USE_LNBIAS:
                        emit_pair_add(0, ot, mt)
                    if (ot % 3 == 2) and out_q:
                        s_, d_, hc_ = out_q.pop(0)
                        emit_output_hc(s_, d_, hc_, act_copy=True)
                    pump(dq, 2)
                dq.append(norm_steps(0, 1, mt))
            while out_q:
                s_, d_, hc_ = out_q.pop(0)
                emit_output_hc(s_, d_, hc_, act_copy=True)

            # ---- P4: pass (1,1) mt-major; weave transpose(0,1)+output(0,1),
            # the norm(1,0) chains, then tp/out(1,0) inside the last chunk.
            out_q = [(0, 1, hc) for hc in range(8)]
            for mt in range(NT):
                if mt == 2:
                    emit_transpose(1, 0)
                    out_q.extend((1, 0, hc) for hc in range(8))
                for ot in range(8):
                    emit_aff(1, 1, ot, mt)
                    if mt == 0 and ot == 1:
                        pump(dq, 99)   # finish norm(0,1) before transposing it
                        emit_transpose(0, 1)
                        dq.extend(norm_steps(1, 0, mt_) for mt_ in range(NT))
                    if USE_LNBIAS:
                        emit_pair_add(1, ot, mt)
                    if out_q and (mt == 2 or ot % 3 == 2):
                        s_, d_, hc_ = out_q.pop(0)
                        emit_output_hc(s_, d_, hc_, act_copy=(d_ == 0 or hc_ % 2 == 0))
                    pump(dq, 3)
                dq.append(norm_steps(1, 1, mt))
            while out_q:
                s_, d_, hc_ = out_q.pop(0)
                emit_output_hc(s_, d_, hc_, act_copy=True)
            pump(dq, 99)
            emit_transpose(1, 1, act_copy=True)
            for hc in range(8):
                emit_output_hc(1, 1, hc, act_copy=True)
    split_excess_waits(nc)
    return nc


_NC_CACHE = {}


def _get_nc(T=T_DEFAULT):
    if T not in _NC_CACHE:
        _NC_CACHE[T] = build_nc(T)
    return _NC_CACHE[T]


def pick_T(inputs):
    mx = np.asarray(inputs["mask_x"])
    my = np.asarray(inputs["mask_y"])
    need = int(max(mx.sum(axis=1).max(), my.sum(axis=1).max())) + MEM
    return max(T_DEFAULT, ((need + 31) // 32) * 32)


def _prep_batch(T, xb, yb, mask_xb, mask_yb, x_memory, y_memory):
    kx = np.flatnonzero(mask_xb != 0)
    ky = np.flatnonzero(mask_yb != 0)
    nkx, nky = len(kx) + MEM, len(ky) + MEM
    assert nkx <= T and nky <= T, f"too many unmasked tokens: {nkx} {nky}"

    Xc = np.zeros((T, HIDDEN), dtype=np.float32)
    Xc[0:MEM] = x_memory
    Xc[MEM:nkx] = xb[kx]
    Yc = np.zeros((T, HIDDEN), dtype=np.float32)
    Yc[0:MEM] = y_memory
    Yc[MEM:nky] = yb[ky]

    import ml_dtypes
    inv_h = np.float32(1.0 / HEADS)

    def pack(tc):
        return np.ascontiguousarray(tc.reshape(8, 128, -1).transpose(1, 0, 2))

    def packmem(mc):
        nt = (T + 127) // 128
        full = np.zeros((nt * 128, HIDDEN), dtype=mc.dtype)
        full[:T] = mc
        return np.ascontiguousarray(full.reshape(nt, 128, HIDDEN).transpose(1, 0, 2))

    return {
        "xT": pack(np.ascontiguousarray(Xc.T)).astype(np.float16),
        "yT": pack(np.ascontiguousarray(Yc.T)).astype(np.float16),
        "xc": packmem((Xc * inv_h).astype(ml_dtypes.bfloat16)),
        "yc": packmem((Yc * inv_h).astype(ml_dtypes.bfloat16)),
        "cor": np.array([T - nkx, T - nky], dtype=np.float32),
    }, (kx, ky, nkx, nky)


def _run_spmd(nc, in_maps, trace=False):
    from concourse.bass_utils import run_bass_kernel_spmd
    return run_bass_kernel_spmd(nc, in_maps, list(range(NCORES)), trace=trace)


def prep_all(inputs, ncores=NCORES):
    T = pick_T(inputs)
    x = np.asarray(inputs["x"], dtype=np.float32)
    y = np.asarray(inputs["y"], dtype=np.float32)
    mask_x = np.asarray(inputs["mask_x"])
    mask_y = np.asarray(inputs["mask_y"])
    Wx = np.asarray(inputs["Wx"], dtype=np.float32)
    Wy = np.asarray(inputs["Wy"], dtype=np.float32)
    x_memory = np.asarray(inputs["x_memory"], dtype=np.float32)
    y_memory = np.asarray(inputs["y_memory"], dtype=np.float32)

    wxT = np.ascontiguousarray(Wx.T.reshape(8, 128, HIDDEN).transpose(1, 0, 2)).astype(np.float16)
    wyT = np.ascontiguousarray(Wy.T.reshape(8, 128, HIDDEN).transpose(1, 0, 2)).astype(np.float16)
    ident = np.eye(128, dtype=np.float32)

    in_maps, scatter = [], []
    for c in range(ncores):
        m = {"wxT": wxT, "wyT": wyT, "ident": ident}
        cors = np.zeros((128, 2 * BPC), dtype=np.float32)
        for s in range(BPC):
            b = c * BPC + s
            piece, info = _prep_batch(T, x[b], y[b], mask_x[b], mask_y[b],
                                      x_memory, y_memory)
            cors[:, 2 * s:2 * s + 2] = piece.pop("cor")[None, :]
            for k, v in piece.items():
                m[f"{k}{s}"] = v
            scatter.append(info)
        m["corr"] = cors
        in_maps.append(m)
    return in_maps, scatter, T


def assemble(inputs, results, scatter, ncores=NCORES):
    x = np.asarray(inputs["x"], dtype=np.float32)
    y = np.asarray(inputs["y"], dtype=np.float32)
    x_memory = np.asarray(inputs["x_memory"], dtype=np.float32)
    y_memory = np.asarray(inputs["y_memory"], dtype=np.float32)
    nb = ncores * BPC
    X_in_Y = np.empty((nb, SEQ, HIDDEN), dtype=np.float32)
    Y_in_X = np.empty((nb, SEQ, HIDDEN), dtype=np.float32)
    for c in range(ncores):
        for s in range(BPC):
            b = c * BPC + s
            kx, ky, nkx, nky = scatter[b]
            xiyT = np.asarray(results[c][f"xiyT{s}"], dtype=np.float32).reshape(HIDDEN, -1)
            yixT = np.asarray(results[c][f"yixT{s}"], dtype=np.float32).reshape(HIDDEN, -1)
            ux = (x_memory.sum(axis=0) + x[b].sum(axis=0)) / np.float32(SEQ + MEM)
            uy = (y_memory.sum(axis=0) + y[b].sum(axis=0)) / np.float32(SEQ + MEM)
            X_in_Y[b] = ux
            X_in_Y[b, ky] = xiyT[:, MEM:nky].T
            Y_in_X[b] = uy
            Y_in_X[b, kx] = yixT[:, MEM:nkx].T
    return X_in_Y, Y_in_X


def run(inputs, trace=False):
    in_maps, scatter, T = prep_all(inputs)
    nc = _get_nc(T)
    res = _run_spmd(nc, in_maps, trace=trace)
    X_in_Y, Y_in_X = assemble(inputs, res.results, scatter)
    return (X_in_Y, Y_in_X), res.exec_time_ns


def kernel(**inputs):
    out, _ = run(inputs)
    return out


# revision 24
# speedup vs baseline: 1.5112x; 1.5112x over previous
"""Trainium2 Bass kernel for nn_MultiHeadAttention_9131100471662.

Cross-attention with memory tokens, dual softmax (rows+columns of the
affinity matrix), head-mean, masked tokens.

v3 strategy:
  - Data-parallel over batch: 16 batches -> 8 cores x 2 batches.
  - Host-side mask compaction (exact), T=288 fixed slots.
  - Per batch, two affinity orientations:
      d=0: e0_h[x,y] = exp(aff), ScalarE singles with accum_out -> den_Y
           (softmax-over-y dens). One-hot PE matvecs partition-sum the
           e0 tiles -> den_X (softmax-over-x dens) -> rcp -> ln(rcp).
      d=1: e1_h[y,x] = exp(aff + ln(rcp_X[y]))  -- the per-partition bias
           PRE-NORMALIZES the tiles, so the head-sum is a plain
           tensor_tensor add tree on DVE (2x perf mode) instead of the
           1x scalar_tensor_tensor chain. No accum_out needed.
    (USE_LNBIAS=False falls back to paired d=1 exps + STT chains.)
  - d=0 normalize: DVE scalar_tensor_tensor chains with rcp_Y.
  - Emission order = engine priority: ScalarE (exp) is the global
    bottleneck (~100us); PE work (projections of the next pass, dens
    matvecs, transposes, output matmuls) is woven between affinity
    groups at fine grain so neither ScalarE nor PE idles (PE drops to a
    1.2 GHz p-state when idle, doubling matmul cost).
  - PSUM rings: "af2" [128,2,512] f32 x2 (affinity pairs), "pj" [128,512]
    f32 x2 (projection singles + output matmuls), "tp" [128,2,512] bf16 x1
    (transposes), "big" [128,512] f32 x1 (dens matvec accumulator) = 8 banks.
  - 1/HEADS head-mean folded into host-side memory matrices.
"""

import numpy as np

import bass_rust
import concourse.bass as bass
import concourse.mybir as mybir
from concourse.tile import TileContext

B = 16
SEQ = 512
HIDDEN = 1024
HEADS = 16
MEM = 2
DH = 64
NCORES = 8
BPC = 2
T_DEFAULT = 288
F32 = mybir.dt.float32
BF16 = mybir.dt.bfloat16
F16 = mybir.dt.float16

PROJ_DT = F16
E_DT = BF16
A_DT = BF16
MEM_DT = BF16
OUT_DT = BF16

USE_LNBIAS = False


def _chunks(T):
    out = []
    o = 0
    while o < T:
        w = min(128, T - o)
        out.append((o, w))
        o += w
    return out


def _patched_drain_and_barrier(self, tick_clock, wait_clock):
    # Workaround: this walrus build rejects a Drain carrying >1 sem waits.
    nc = self.nc
    drain_inst = nc.sync.drain()
    wait_clock.add_sem_waits(
        drain_inst.ins, bass_rust.ScopedClock({None: tick_clock.global_clock})
    )
    inst = drain_inst.ins
    si = inst.sync_info
    waits = list(si.on_wait) if si and si.on_wait else []
    si.on_wait = []
    name2sem = {s.name: s for s in self.sems.allocated().values()}
    for w in waits:
        assert w.wait_mode == "sem-ge-imm", w
        nc.sync.wait_ge(name2sem[w.ant_name], w.wait_value)
    nc.all_engine_barrier()
    popped = nc._tile_sem_poison_stack.pop()
    assert popped is self._sem_poison
    nc.clear_and_free_semaphores(list(self.sems.allocated().values()))
    nc.all_engine_barrier()


TileContext._drain_and_barrier = _patched_drain_and_barrier


def split_excess_waits(nc, cap=1):
    """Hoist >cap sem waits per instruction onto injected NoOps."""
    for f in nc.m.functions:
        for bb in f.blocks:
            newlist, changed = [], False
            for inst in bb.instructions:
                si = inst.sync_info
                waits = list(si.on_wait) if si and si.on_wait else []
                if len(waits) > cap:
                    changed = True
                    for w in waits[:-cap]:
                        nop = mybir.InstNoOp(
                            name=nc.get_next_instruction_name(), ins=[], outs=[])
                        nop.engine = inst.engine
                        nop.sync_info = mybir.SyncInfo(on_wait=[w], on_update=[])
                        nc.register_instruction(nop, overwrite=True)
                        newlist.append(nop)
                    si.on_wait = waits[-cap:]
                newlist.append(inst)
            if changed:
                bb.instructions = newlist


def build_nc(T=T_DEFAULT):
    CH = _chunks(T)
    NT = len(CH)
    nc = bass.Bass()
    p = {}
    p["wxT"] = nc.declare_dram_parameter("wxT", [128, 8, HIDDEN], PROJ_DT, isOutput=False)
    p["wyT"] = nc.declare_dram_parameter("wyT", [128, 8, HIDDEN], PROJ_DT, isOutput=False)
    p["ident"] = nc.declare_dram_parameter("ident", [128, 128], F32, isOutput=False)
    for s in range(BPC):
        p[f"xT{s}"] = nc.declare_dram_parameter(f"xT{s}", [128, 8, T], PROJ_DT, isOutput=False)
        p[f"yT{s}"] = nc.declare_dram_parameter(f"yT{s}", [128, 8, T], PROJ_DT, isOutput=False)
        p[f"xc{s}"] = nc.declare_dram_parameter(f"xc{s}", [128, NT, HIDDEN], MEM_DT, isOutput=False)
        p[f"yc{s}"] = nc.declare_dram_parameter(f"yc{s}", [128, NT, HIDDEN], MEM_DT, isOutput=False)
    p["corr"] = nc.declare_dram_parameter("corr", [128, 2 * BPC], F32, isOutput=False)
    for s in range(BPC):
        # outputs transposed, per-128-column chunk: [hc, part, T]
        p[f"xiyT{s}"] = nc.declare_dram_parameter(f"xiyT{s}", [8, 128, T], OUT_DT, isOutput=True)
        p[f"yixT{s}"] = nc.declare_dram_parameter(f"yixT{s}", [8, 128, T], OUT_DT, isOutput=True)

    with TileContext(nc, pool_alloc_mode="queue") as tc:
        import contextlib
        with contextlib.ExitStack() as ctx:
            cpool = ctx.enter_context(tc.tile_pool(name="consts", bufs=1))
            projpool = ctx.enter_context(tc.tile_pool(name="proj", bufs=1))
            psum = ctx.enter_context(tc.tile_pool(name="psum", bufs=1, space="PSUM"))
            epool = ctx.enter_context(tc.tile_pool(name="epool", bufs=1))
            apool = ctx.enter_context(tc.tile_pool(name="apool", bufs=1))
            smallpool = ctx.enter_context(tc.tile_pool(name="small", bufs=1))
            xcpool = ctx.enter_context(tc.tile_pool(name="xcpool", bufs=1))
            w_scope = contextlib.ExitStack()
            wpool = w_scope.enter_context(tc.tile_pool(name="weights", bufs=1))
            inpool = w_scope.enter_context(tc.tile_pool(name="inputs", bufs=1))

            _c = {}

            def preload_exp_table():
                t_ = cpool.tile([128, 16], F32, name="dummy")
                nc.vector.memset(t_[:, :], 0.0)
                nc.scalar.activation(t_[:, :], t_[:, :],
                                     mybir.ActivationFunctionType.Exp)
                if USE_LNBIAS:
                    nc.vector.memset(t_[:, :], 1.0)
                    nc.scalar.activation(t_[:, :], t_[:, :],
                                         mybir.ActivationFunctionType.Ln)
                # PE p-state warmup during the input-DMA wait: ~3us of dummy
                # matmuls ramps the tensor engine to 2.4 GHz before the first
                # projection (cold PE runs at 0.65-1.2 GHz).
                wb = cpool.tile([128, 448], BF16, name="warm")
                nc.vector.memset(wb[:, :], 0.0)
                wp = psum.tile([128, 512], F32, name="warm_ps", tag="pj", bufs=2)
                for i in range(18):
                    nc.tensor.matmul(wp[0:64, 0:448], wb[:, 0:64], wb[:, :],
                                     start=(i == 0), stop=(i == 17))
                nc.vector.tensor_copy(t_[0:1, 0:1], wp[0:1, 0:1])

            def load_consts():
                ident_sb = cpool.tile([128, 128], F32, name="ident_sb")
                nc.sync.dma_start(out=ident_sb[:, :], in_=p["ident"][:, :])
                identb_sb = cpool.tile([128, 128], A_DT, name="identb_sb")
                nc.vector.tensor_copy(identb_sb[:, :], ident_sb[:, :])
                corr_sb = cpool.tile([128, 2 * BPC], F32, name="corr_sb")
                nc.sync.dma_start(out=corr_sb[:, :], in_=p["corr"][:, :])
                for s_ in range(BPC):
                    for ci, side in enumerate(("x", "y")):
                        _c[f"cor{side}{s_}"] = corr_sb[:, 2 * s_ + ci:2 * s_ + ci + 1]
                oh = cpool.tile([128, HEADS, HEADS], E_DT, name="onehot_sb")
                nc.vector.memset(oh[:, :, :], 0.0)
                for h in range(HEADS):
                    nc.vector.memset(oh[:, h, h:h + 1], 1.0)
                _c["onehot"] = oh
                _c["ident"], _c["identb"] = ident_sb, identb_sb

            w_sb, tT_sb = {}, {}
            w_first = {}

            def load_w_first(side, ot=0):
                wname = "wxT" if side == "x" else "wyT"
                t_ = wpool.tile([128, 8, 128], PROJ_DT, name=f"wf{side}{ot}",
                                tag=f"wf{side}{ot}")
                nc.sync.dma_start(out=t_[:, :, :],
                                  in_=p[wname][:, :, ot * 128:(ot + 1) * 128])
                w_first[(side, ot)] = t_

            def load_w_half(side, hf):
                wname = "wxT" if side == "x" else "wyT"
                t_ = wpool.tile([128, 8, 512], PROJ_DT, name=f"w{side}{hf}",
                                tag=f"w{side}{hf}")
                nc.sync.dma_start(out=t_[:, :, :],
                                  in_=p[wname][:, :, hf * 512:(hf + 1) * 512])
                for kt in range(8):
                    w_sb[(side, kt, hf)] = t_[:, kt, :]

            def load_tT(s, side):
                t_ = inpool.tile([128, 8, T], PROJ_DT, name=f"tT{side}{s}",
                                 tag=f"tT{side}{s}")
                nc.sync.dma_start(out=t_[:, :, :], in_=p[f"{side}T{s}"][:, :, :])
                for kt in range(8):
                    tT_sb[(s, side, kt)] = t_[:, kt, :]

            def load_mem(s):
                for side in ("x", "y"):
                    t_ = xcpool.tile([128, NT, HIDDEN], MEM_DT,
                                     name=f"mem{side}{s}", tag=f"mem{side}", bufs=1)
                    nc.sync.dma_start(out=t_[:, :, :], in_=p[f"{side}c{s}"][:, :, :])
                    for kt in range(NT):
                        mem_sb[(s, side, kt)] = t_[:, kt, :]

            proj_sb, mem_sb = {}, {}
            e_sb, den_sb, rcp_sb, lnr_sb, a_sb, at_sb, rs_ps = {}, {}, {}, {}, {}, {}, {}

            def _w_slice(side, ot, kt):
                if (side, ot) in w_first:
                    return w_first[(side, ot)][:, kt, :]
                return w_sb[(side, kt, ot // 4)][:, (ot % 4) * 128:(ot % 4 + 1) * 128]

            def emit_proj_single(s, side, ot):
                ptf = psum.tile([128, 512], F32, name="pj_ps", tag="pj", bufs=2)
                pt = ptf[:, 0:T]
                for kt in range(8):
                    nc.tensor.matmul(
                        pt, _w_slice(side, ot, kt), tT_sb[(s, side, kt)][:, :],
                        start=(kt == 0), stop=(kt == 7),
                    )
                st = projpool.tile([128, T], PROJ_DT, name=f"pj{s}{side}{ot}",
                                   tag=f"pj{s}{side}{ot}")
                nc.vector.tensor_copy(st[:, :], pt)
                proj_sb[(s, side, ot)] = st

            def alloc_den(s):
                for mt in range(NT):
                    den_sb[(s, 0, mt)] = smallpool.tile(
                        [128, HEADS], F32, name=f"den{s}0{mt}", tag=f"den0{mt}", bufs=2)

            def emit_aff(s, d, ot, mt):
                """Affinity head-pair (2ot, 2ot+1), stationary chunk mt.
                d=0: singles + accum_out (den_Y).  d=1 (LNBIAS): singles with
                bias=ln(rcp_X); else one paired exp."""
                stat_side, mov_side = ("x", "y") if d == 0 else ("y", "x")
                lo_c, w_c = CH[mt]
                stat = proj_sb[(s, stat_side, ot)]
                mov = proj_sb[(s, mov_side, ot)]
                af = psum.tile([128, 2, 512], F32, name="af_ps", tag="af2", bufs=2)
                for half in range(2):
                    lo = 64 * half
                    nc.tensor.matmul(
                        af[0:w_c, half, 0:T],
                        stat[lo:lo + 64, lo_c:lo_c + w_c],
                        mov[lo:lo + 64, :],
                        start=True, stop=True,
                    )
                ep = epool.tile([128, 2, T], E_DT, name="e_t", tag=f"e{d}",
                                bufs=(48 if d == 0 else 27))
                if d == 0:
                    for half in range(2):
                        h = 2 * ot + half
                        nc.scalar.activation(
                            ep[0:w_c, half, :], af[0:w_c, half, 0:T],
                            mybir.ActivationFunctionType.Exp,
                            accum_out=den_sb[(s, 0, mt)][0:w_c, h:h + 1],
                        )
                elif USE_LNBIAS:
                    for half in range(2):
                        h = 2 * ot + half
                        nc.scalar.activation(
                            ep[0:w_c, half, :], af[0:w_c, half, 0:T],
                            mybir.ActivationFunctionType.Exp,
                            bias=lnr_sb[(s, mt)][0:w_c, h:h + 1],
                        )
                else:
                    nc.scalar.activation(
                        ep[0:w_c, :, :], af[0:w_c, :, 0:T],
                        mybir.ActivationFunctionType.Exp,
                    )
                e_sb[(s, d, 2 * ot, mt)] = ep[:, 0, :]
                e_sb[(s, d, 2 * ot + 1, mt)] = ep[:, 1, :]

            def emit_mv(s, heads):
                """One-hot matvecs: partition-sum e0 tiles of `heads` over all
                chunks into rs[16, T] (accumulating across calls)."""
                rs = rs_ps.setdefault(
                    s, psum.tile([128, 512], F32, name="rs_ps", tag="big", bufs=1))
                for h in heads:
                    for chunk, (klo, kw) in enumerate(CH):
                        nc.tensor.matmul(
                            rs[0:16, 0:T],
                            _c["onehot"][0:kw, h, :],
                            e_sb[(s, 0, h, chunk)][0:kw, :],
                            start=(h == 0 and chunk == 0),
                            stop=(h == HEADS - 1 and chunk == NT - 1),
                            skip_group_check=True,
                        )

            def emit_mv_finalize(s):
                """rs -> per-y-chunk rcp_X (and ln(rcp_X) for the d=1 bias)."""
                rs = rs_ps[s]
                rssb = smallpool.tile([16, T], F32, name=f"rssb{s}", tag="rssb", bufs=2)
                nc.vector.tensor_copy(rssb[:, :], rs[0:16, 0:T])
                corr = _c[f"corx{s}"]
                for mt, (lo_c, w_c) in enumerate(CH):
                    dps = psum.tile([128, 512], F32, name="dps", tag="big", bufs=1)
                    nc.tensor.transpose(dps[0:w_c, 0:16], rssb[:, lo_c:lo_c + w_c],
                                        _c["ident"][0:16, 0:16])
                    nc.vector.tensor_scalar_sub(
                        dps[0:w_c, 0:16], dps[0:w_c, 0:16], corr[0:w_c, 0:1])
                    rcp = smallpool.tile([128, HEADS], F32, name=f"rcp{s}1{mt}",
                                         tag=f"rcp1{mt}", bufs=2)
                    nc.vector.reciprocal(rcp[0:w_c, :], dps[0:w_c, 0:16])
                    rcp_sb[(s, 1, mt)] = rcp
                    if USE_LNBIAS:
                        lnr = smallpool.tile([128, HEADS], F32, name=f"lnr{s}{mt}",
                                             tag=f"lnr{mt}", bufs=2)
                        nc.scalar.activation(lnr[0:w_c, :], rcp[0:w_c, :],
                                             mybir.ActivationFunctionType.Ln)
                        lnr_sb[(s, mt)] = lnr

            def norm_steps(s, d, mt):
                """Generator of DVE steps for a normalize chain (sliced so
                the emission loop can weave copies between steps)."""
                lo_c, w_c = CH[mt]
                if d == 0:
                    den = den_sb[(s, 0, mt)]
                    corr = _c[f"cory{s}"]
                    nc.vector.tensor_scalar_sub(den[0:w_c, :], den[0:w_c, :],
                                                corr[0:w_c, 0:1])
                    rcp = smallpool.tile([128, HEADS], F32, name=f"rcp{s}0{mt}",
                                         tag=f"rcp0{mt}", bufs=2)
                    nc.vector.reciprocal(rcp[0:w_c, :], den[0:w_c, :])
                else:
                    rcp = rcp_sb[(s, 1, mt)]
                es = [e_sb[(s, d, h, mt)] for h in range(HEADS)]
                a = apool.tile([128, T], A_DT, name=f"a{s}{d}{mt}", tag=f"a{d}{mt}", bufs=2)
                nc.vector.tensor_scalar_mul(a[0:w_c, :], es[0][0:w_c, :], rcp[0:w_c, 0:1])
                a_sb[(s, d, mt)] = a
                yield
                for h in range(1, HEADS):
                    nc.vector.scalar_tensor_tensor(
                        out=a[0:w_c, :], in0=es[h][0:w_c, :],
                        scalar=rcp[0:w_c, h:h + 1], in1=a[0:w_c, :],
                        op0=mybir.AluOpType.mult, op1=mybir.AluOpType.add)
                    if h % 4 == 0:
                        yield

            def emit_pair_add(s, ot, mt):
                """Tree level 0 for d=1: e[2ot] += e[2ot+1] (2x-mode add)."""
                lo_c, w_c = CH[mt]
                a = e_sb[(s, 1, 2 * ot, mt)]
                b = e_sb[(s, 1, 2 * ot + 1, mt)]
                nc.vector.tensor_tensor(out=a[0:w_c, :], in0=a[0:w_c, :],
                                        in1=b[0:w_c, :], op=mybir.AluOpType.add)

            def emit_tree_tail(s, mt):
                lo_c, w_c = CH[mt]
                if USE_LNBIAS:
                    for step in (2, 4, 8):
                        for h0 in range(0, HEADS, 2 * step):
                            a = e_sb[(s, 1, h0, mt)]
                            b = e_sb[(s, 1, h0 + step, mt)]
                            nc.vector.tensor_tensor(
                                out=a[0:w_c, :], in0=a[0:w_c, :],
                                in1=b[0:w_c, :], op=mybir.AluOpType.add)
                    a_sb[(s, 1, mt)] = e_sb[(s, 1, 0, mt)]
                else:
                    rcp = rcp_sb[(s, 1, mt)]
                    es = [e_sb[(s, 1, h, mt)] for h in range(HEADS)]
                    a = apool.tile([128, T], A_DT, name=f"a{s}1{mt}", tag=f"a1{mt}", bufs=2)
                    nc.vector.tensor_scalar_mul(a[0:w_c, :], es[0][0:w_c, :], rcp[0:w_c, 0:1])
                    for h in range(1, HEADS):
                        nc.vector.scalar_tensor_tensor(
                            out=a[0:w_c, :], in0=es[h][0:w_c, :],
                            scalar=rcp[0:w_c, h:h + 1], in1=a[0:w_c, :],
                            op0=mybir.AluOpType.mult, op1=mybir.AluOpType.add)
                    a_sb[(s, 1, mt)] = a

            def emit_transpose(s, d, act_copy=False):
                cpy = nc.scalar.copy if act_copy else nc.vector.tensor_copy
                slotA = psum.tile([128, 2, 512], A_DT, name="tp_ps", tag="tp", bufs=1)
                for kt in range(2):
                    klo, kw = CH[kt]
                    for mt, (mlo, mw) in enumerate(CH):
                        nc.tensor.transpose(
                            slotA[:, kt, :][0:kw, mlo:mlo + mw],
                            a_sb[(s, d, mt)][0:mw, klo:klo + kw],
                            _c["identb"][0:mw, 0:mw],
                        )
                stAB = apool.tile([128, 2, T], A_DT, name=f"atp{s}{d}", tag=f"atp{d}", bufs=2)
                cpy(stAB[:, :, :], slotA[:, :, 0:T])
                slotB = psum.tile([128, 2, 512], A_DT, name="tp_ps", tag="tp", bufs=1)
                klo, kw = CH[2]
                for mt, (mlo, mw) in enumerate(CH):
                    nc.tensor.transpose(
                        slotB[:, 0, :][0:kw, mlo:mlo + mw],
                        a_sb[(s, d, mt)][0:mw, klo:klo + kw],
                        _c["identb"][0:mw, 0:mw],
                    )
                st2 = apool.tile([128, T], A_DT, name=f"at2{s}{d}", tag=f"at2{d}", bufs=2)
                cpy(st2[0:kw, :], slotB[0:kw, 0, 0:T])
                at_sb[(s, d, 0)] = stAB[:, 0, :]
                at_sb[(s, d, 1)] = stAB[:, 1, :]
                at_sb[(s, d, 2)] = st2

            def emit_output_hc(s, d, hc, act_copy=False):
                cpy = nc.scalar.copy if act_copy else nc.vector.tensor_copy
                rhs_side, oname = (("y", f"yixT{s}"), ("x", f"xiyT{s}"))[d]
                opf = psum.tile([128, 512], F32, name="out_ps", tag="pj", bufs=2)
                op = opf[:, 0:T]
                for kt, (klo, kw) in enumerate(CH):
                    nc.tensor.matmul(
                        op,
                        mem_sb[(s, rhs_side, kt)][0:kw, hc * 128:(hc + 1) * 128],
                        at_sb[(s, d, kt)][0:kw, :],
                        start=(kt == 0), stop=(kt == NT - 1),
                    )
                ost = smallpool.tile([128, T], OUT_DT, name="ost", tag="ost", bufs=4)
                cpy(ost[:, :], op)
                nc.sync.dma_start(out=p[oname][hc, :, :], in_=ost[:, :])

            # ================= schedule =================
            load_tT(0, "x")
            load_tT(0, "y")
            load_w_first("x")
            load_w_first("y")
            preload_exp_table()
            alloc_den(0)

            # ---- P1: pass (0,0) ot-major; weave proj(0) lag-1, mv(0) lag-1
            emit_proj_single(0, "x", 0)
            emit_proj_single(0, "y", 0)
            load_w_half("x", 0)
            load_w_half("y", 0)
            for ot in range(8):
                emit_aff(0, 0, ot, 0)
                emit_aff(0, 0, ot, 1)
                if ot < 7:
                    emit_proj_single(0, "x", ot + 1)
                emit_aff(0, 0, ot, 2)
                if ot < 7:
                    emit_proj_single(0, "y", ot + 1)
                else:
                    emit_proj_single(1, "x", 0)
                    emit_proj_single(1, "y", 0)
                if ot == 0:
                    load_w_half("x", 1)
                    load_w_half("y", 1)
                    load_consts()
                if ot == 2:
                    load_tT(1, "x")
                    load_tT(1, "y")
                if ot == 4:
                    load_mem(0)
                if ot == 6:
                    load_mem(1)
                if ot > 0:
                    emit_mv(0, (2 * ot - 2, 2 * ot - 1))
            emit_mv(0, (14, 15))
            emit_mv_finalize(0)

            # ---- P2: pass (1,0) ot-major; weave proj(1) lag-1, mv(1) lag-1,
            # and the norm(0,0) STT chains (sliced) on DVE.
            alloc_den(1)
            n00 = [norm_steps(0, 0, mt) for mt in range(NT)]

            def pump(gens, k=1):
                for _ in range(k):
                    for g in list(gens):
                        try:
                            next(g)
                            break
                        except StopIteration:
                            gens.remove(g)

            for ot in range(8):
                emit_aff(1, 0, ot, 0)
                emit_aff(1, 0, ot, 1)
                if ot < 7:
                    emit_proj_single(1, "x", ot + 1)
                emit_aff(1, 0, ot, 2)
                if ot < 7:
                    emit_proj_single(1, "y", ot + 1)
                if ot > 0:
                    emit_mv(1, (2 * ot - 2, 2 * ot - 1))
                pump(n00, 2)
            emit_mv(1, (14, 15))
            emit_mv_finalize(1)
            pump(n00, 99)
            w_scope.close()

            # ---- P3: pass (0,1) mt-major; weave transpose(0,0)+output(0,0)
            # (copies on the then-slack ScalarE) and the norm(0,1) chains
            # with a one-chunk lag.
            dq = []
            out_q = [(0, 0, hc) for hc in range(8)]
            for mt in range(NT):
                for ot in range(8):
                    emit_aff(0, 1, ot, mt)
                    if mt == 0 and ot == 1:
                        emit_transpose(0, 0)
                    if USE_LNBIAS:
                        emit_pair_add(0, ot, mt)
                    if (ot % 3 == 2) and out_q:
                        s_, d_, hc_ = out_q.pop(0)
                        emit_output_hc(s_, d_, hc_, act_copy=True)
                    pump(dq, 2)
                dq.append(norm_steps(0, 1, mt))
            while out_q:
                s_, d_, hc_ = out_q.pop(0)
                emit_output_hc(s_, d_, hc_, act_copy=True)

            # ---- P4: pass (1,1) mt-major; weave transpose(0,1)+output(0,1),
            # the norm(1,0) chains, then tp/out(1,0) inside the last chunk.
            out_q = [(0, 1, hc) for hc in range(8)]
            for mt in range(NT):
                if mt == 2:
                    emit_transpose(1, 0)
                    out_q.extend((1, 0, hc) for hc in range(8))
                for ot in range(8):
                    emit_aff(1, 1, ot, mt)
                    if mt == 0 and ot == 1:
                        pump(dq, 99)   # finish norm(0,1) before transposing it
                        emit_transpose(0, 1)
                        dq.extend(norm_steps(1, 0, mt_) for mt_ in range(NT))
                    if USE_LNBIAS:
                        emit_pair_add(1, ot, mt)
                    if out_q and (mt == 2 or ot % 3 == 2):
                        s_, d_, hc_ = out_q.pop(0)
                        emit_output_hc(s_, d_, hc_, act_copy=(d_ == 0 or hc_ % 2 == 0))
                    pump(dq, 3)
                dq.append(norm_steps(1, 1, mt))
            while out_q:
                s_, d_, hc_ = out_q.pop(0)
                emit_output_hc(s_, d_, hc_, act_copy=True)
            pump(dq, 99)
            emit_transpose(1, 1, act_copy=True)
            for hc in range(8):
                emit_output_hc(1, 1, hc, act_copy=True)
    split_excess_waits(nc)
    return nc


_NC_CACHE = {}


def _get_nc(T=T_DEFAULT):
    if T not in _NC_CACHE:
        _NC_CACHE[T] = build_nc(T)
    return _NC_CACHE[T]


def pick_T(inputs):
    mx = np.asarray(inputs["mask_x"])
    my = np.asarray(inputs["mask_y"])
    need = int(max(mx.sum(axis=1).max(), my.sum(axis=1).max())) + MEM
    return max(T_DEFAULT, ((need + 31) // 32) * 32)


def _prep_batch(T, xb, yb, mask_xb, mask_yb, x_memory, y_memory):
    kx = np.flatnonzero(mask_xb != 0)
    ky = np.flatnonzero(mask_yb != 0)
    nkx, nky = len(kx) + MEM, len(ky) + MEM
    assert nkx <= T and nky <= T, f"too many unmasked tokens: {nkx} {nky}"

    Xc = np.zeros((T, HIDDEN), dtype=np.float32)
    Xc[0:MEM] = x_memory
    Xc[MEM:nkx] = xb[kx]
    Yc = np.zeros((T, HIDDEN), dtype=np.float32)
    Yc[0:MEM] = y_memory
    Yc[MEM:nky] = yb[ky]

    import ml_dtypes
    inv_h = np.float32(1.0 / HEADS)

    def pack(tc):
        return np.ascontiguousarray(tc.reshape(8, 128, -1).transpose(1, 0, 2))

    def packmem(mc):
        nt = (T + 127) // 128
        full = np.zeros((nt * 128, HIDDEN), dtype=mc.dtype)
        full[:T] = mc
        return np.ascontiguousarray(full.reshape(nt, 128, HIDDEN).transpose(1, 0, 2))

    return {
        "xT": pack(np.ascontiguousarray(Xc.T)).astype(np.float16),
        "yT": pack(np.ascontiguousarray(Yc.T)).astype(np.float16),
        "xc": packmem((Xc * inv_h).astype(ml_dtypes.bfloat16)),
        "yc": packmem((Yc * inv_h).astype(ml_dtypes.bfloat16)),
        "cor": np.array([T - nkx, T - nky], dtype=np.float32),
    }, (kx, ky, nkx, nky)


def _run_spmd(nc, in_maps, trace=False):
    from concourse.bass_utils import run_bass_kernel_spmd
    return run_bass_kernel_spmd(nc, in_maps, list(range(NCORES)), trace=trace)


def prep_all(inputs, ncores=NCORES):
    T = pick_T(inputs)
    x = np.asarray(inputs["x"], dtype=np.float32)
    y = np.asarray(inputs["y"], dtype=np.float32)
    mask_x = np.asarray(inputs["mask_x"])
    mask_y = np.asarray(inputs["mask_y"])
    Wx = np.asarray(inputs["Wx"], dtype=np.float32)
    Wy = np.asarray(inputs["Wy"], dtype=np.float32)
    x_memory = np.asarray(inputs["x_memory"], dtype=np.float32)
    y_memory = np.asarray(inputs["y_memory"], dtype=np.float32)

    wxT = np.ascontiguousarray(Wx.T.reshape(8, 128, HIDDEN).transpose(1, 0, 2)).astype(np.float16)
    wyT = np.ascontiguousarray(Wy.T.reshape(8, 128, HIDDEN).transpose(1, 0, 2)).astype(np.float16)
    ident = np.eye(128, dtype=np.float32)

    in_maps, scatter = [], []
    for c in range(ncores):
        m = {"wxT": wxT, "wyT": wyT, "ident": ident}
        cors = np.zeros((128, 2 * BPC), dtype=np.float32)
        for s in range(BPC):
            b = c * BPC + s
            piece, info = _prep_batch(T, x[b], y[b], mask_x[b], mask_y[b],
                                      x_memory, y_memory)
            cors[:, 2 * s:2 * s + 2] = piece.pop("cor")[None, :]
            for k, v in piece.items():
                m[f"{k}{s}"] = v
            scatter.append(info)
        m["corr"] = cors
        in_maps.append(m)
    return in_maps, scatter, T


def assemble(inputs, results, scatter, ncores=NCORES):
    x = np.asarray(inputs["x"], dtype=np.float32)
    y = np.asarray(inputs["y"], dtype=np.float32)
    x_memory = np.asarray(inputs["x_memory"], dtype=np.float32)
    y_memory = np.asarray(inputs["y_memory"], dtype=np.float32)
    nb = ncores * BPC
    X_in_Y = np.empty((nb, SEQ, HIDDEN), dtype=np.float32)
    Y_in_X = np.empty((nb, SEQ, HIDDEN), dtype=np.float32)
    for c in range(ncores):
        for s in range(BPC):
            b = c * BPC + s
            kx, ky, nkx, nky = scatter[b]
            xiyT = np.asarray(results[c][f"xiyT{s}"], dtype=np.float32).reshape(HIDDEN, -1)
            yixT = np.asarray(results[c][f"yixT{s}"], dtype=np.float32).reshape(HIDDEN, -1)
            ux = (x_memory.sum(axis=0) + x[b].sum(axis=0)) / np.float32(SEQ + MEM)
            uy = (y_memory.sum(axis=0) + y[b].sum(axis=0)) / np.float32(SEQ + MEM)
            X_in_Y[b] = ux
            X_in_Y[b, ky] = xiyT[:, MEM:nky].T
            Y_in_X[b] = uy
            Y_in_X[b, kx] = yixT[:, MEM:nkx].T
    return X_in_Y, Y_in_X


def run(inputs, trace=False):
    in_maps, scatter, T = prep_all(inputs)
    nc = _get_nc(T)
    res = _run_spmd(nc, in_maps, trace=trace)
    X_in_Y, Y_in_X = assemble(inputs, res.results, scatter)
    return (X_in_Y, Y_in_X), res.exec_time_ns


def kernel(**inputs):
    out, _ = run(inputs)
    return out
